# revision 5
# baseline (speedup 1.0000x reference)
"""Trainium2 Bass kernel for nn_ConvOnTree (gnn_message_passing).

Computation (reference):
    selected = points[indices]                      # [N, 81, 3]
    dist     = sum((selected - selected[:, :1])**2, -1) + 1
    data     = concat(selected, dist[..., None])    # [N, 81, 4]
    out      = einsum('njc,cjl->nl', dw * data, weight) + bias

Strategy: data-parallel over N across 8 NeuronCores. Each core holds a
host-precomputed feature table h = [x, y, z, 1+|x|^2] ([N, 4], 16B rows) in
HBM and gathers neighbor rows with per-tile vector-indirect DMAs (128 rows,
one per partition, per instruction — the SWDGE indirect1d ucode's hard limit;
multi-row-per-partition offset APs lower incorrectly and HWDGE engines cannot
execute the indirect opcode). j = 0 is the self neighbor and is copied from
the point tile instead of gathered. The distance feature is reconstructed
algebraically (dist = (1+|q|^2) - 2 q.p + |p|^2) in 3 DVE ops per tile using
a point tile that carries [-2x, -2y, -2z, |p|^2, x, y, z, 1+|p|^2]. The
einsum contracts (j, c) = 324 on PE in three 108-row passes with dw folded
into the weights on the host; bias is added on ACT. Output tiles are written
as [8, 128] and transposed on the host during unsharding.
"""
import sys
import types

sys.path.insert(0, "/opt/trn_rl_repo")
sys.path.insert(0, "/root/.axon_site")

import numpy as np
import concourse.bass as bass
import concourse.mybir as mybir
import concourse.tile as tile
from concourse.vector_clock import ScopedClock
from concourse.bass_utils import run_bass_kernel_spmd
from concourse.masks import make_identity

fp32 = mybir.dt.float32
i32 = mybir.dt.int32

N = 500000
K = 81
OUT = 8
NCORES = 8
PPC = N // NCORES            # 62500 points per core
TILE = 128
NT = (PPC + TILE - 1) // TILE  # 489 tiles per core
PADPC = NT * TILE            # 62592 padded points per core


def _patched_drain_and_barrier(self, tick_clock, wait_clock):
    # This walrus build's CTRL_NO struct accepts too few sync waits for the
    # tile tail drain; spread the waits across preceding SP nops.
    nops = [self.nc.sync.nop() for _ in range(30)]
    drain_inst = self.nc.sync.drain()
    wait_clock.add_sem_waits(
        drain_inst.ins, ScopedClock({None: tick_clock.global_clock})
    )
    waits = list(drain_inst.ins.sync_info.on_wait) if drain_inst.ins.sync_info else []
    if len(waits) > 1:
        drain_inst.ins.sync_info.on_wait = waits[:1]
        for w, nop in zip(waits[1:], nops):
            si = nop.ins.sync_info
            if si is None:
                nop.ins.sync_info = mybir.SyncInfo(on_wait=[w], on_update=[])
            else:
                si.on_wait.append(w)
    self.nc.all_engine_barrier()
    popped = self.nc._tile_sem_poison_stack.pop()
    assert popped is self._sem_poison
    self.nc.clear_and_free_semaphores(list(self.sems.allocated().values()))
    self.nc.all_engine_barrier()


tile.TileContext._drain_and_barrier = _patched_drain_and_barrier


def _install_ntff_hook():
    # The image's antenv lacks axon_hooks; register the ctypes NTFF hook so
    # trace=True can report HW exec time. Harmless if tracing is never used.
    try:
        from trn_agent_boot.trn_boot import _ntff_profile_via_ctypes

        hook = _ntff_profile_via_ctypes("/opt/axon/libaxon_pjrt.so")
        mod = types.ModuleType("antenv.axon_hooks")
        mod.get_axon_ntff_profile_hook = lambda: hook
        import antenv  # noqa: F401

        sys.modules["antenv.axon_hooks"] = mod
    except Exception:
        pass


_install_ntff_hook()


MAX_WAITS = 1  # this walrus build encodes only one sync wait per instruction


def split_excess_waits(nc):
    """Move sync waits beyond MAX_WAITS onto same-engine InstNoOp carriers
    inserted immediately before the over-limit instruction."""
    n_split = 0
    for f in nc.m.functions:
        for b in f.blocks:
            new_insts = []
            for inst in b.instructions:
                si = inst.sync_info
                if si is not None and si.on_wait and len(si.on_wait) > MAX_WAITS:
                    waits = list(si.on_wait)
                    for k, w in enumerate(waits[MAX_WAITS:]):
                        nop = mybir.InstNoOp(
                            name=f"{inst.name}-wsplit{k}", ins=[], outs=[])
                        nop.engine = inst.engine
                        nop.sync_info = mybir.SyncInfo(on_wait=[w], on_update=[])
                        new_insts.append(nop)
                        n_split += 1
                    si.on_wait = waits[:MAX_WAITS]
                new_insts.append(inst)
            if len(new_insts) != len(b.instructions):
                b.instructions[:] = new_insts
    return n_split


def build_program():
    nc = bass.Bass("TRN2", target_bir_lowering=False, debug=False,
                   num_devices=NCORES)
    # h table: [x, y, z, 1 + |x|^2] per point, 16B rows
    table = nc.dram_tensor("table", [N, 4], fp32, kind="ExternalInput")
    idx_in = nc.dram_tensor("idx", [NT * TILE, K], i32, kind="ExternalInput")
    # per-tile points: [-2x, -2y, -2z, |p|^2, x, y, z, 1+|p|^2]
    pt_in = nc.dram_tensor("ptile", [NT * TILE, 8], fp32, kind="ExternalInput")
    w2_in = nc.dram_tensor("w2", [324, OUT], fp32, kind="ExternalInput")
    bias_in = nc.dram_tensor("bias", [OUT, 1], fp32, kind="ExternalInput")
    out_d = nc.dram_tensor("out", [NT * OUT, TILE], fp32, kind="ExternalOutput")

    with tile.TileContext(nc) as tc:
        with (
            tc.tile_pool(name="const", bufs=1) as cpool,
            tc.tile_pool(name="work", bufs=6) as wpool,
            tc.tile_pool(name="ps", bufs=3, space="PSUM") as ppool,
            tc.tile_pool(name="pso", bufs=2, space="PSUM") as opool,
        ):
            ident = cpool.tile([128, 128], fp32, name="ident")
            make_identity(nc, ident[:])
            w2_tiles = []
            for p in range(3):
                w2p = cpool.tile([108, OUT], fp32, name=f"w2_{p}")
                nc.sync.dma_start(w2p[:], w2_in[108 * p:108 * (p + 1), :])
                w2_tiles.append(w2p)
            bias_t = cpool.tile([OUT, 1], fp32, name="bias_t")
            nc.sync.dma_start(bias_t[:], bias_in[:])

            # Preload ALL per-tile indices and point features once: removes
            # 2x489 per-tile HWDGE loads and the idx-DMA waits (and their
            # wait-split NOPs) from the Pool gather chain.
            idx_all = cpool.tile([TILE, NT * K], i32, name="idx_all")
            nc.sync.dma_start(
                idx_all[:, :],
                bass.AP(idx_in, 0, [[K, TILE], [TILE * K, NT], [1, K]]))
            pt_all = cpool.tile([TILE, NT * 8], fp32, name="pt_all")
            nc.sync.dma_start(
                pt_all[:, :],
                bass.AP(pt_in, 0, [[8, TILE], [TILE * 8, NT], [1, 8]]))

            for iv in range(NT):
                # gathered tile: [128 pts, 81 nbrs, 4] = [x, y, z, 1+|q|^2]
                gc = wpool.tile([TILE, K, 4], fp32, name="gc")
                # j = 0 is the self neighbor: copy the h-row from pt_all.
                nc.vector.tensor_copy(
                    out=gc[:, 0, 0:4],
                    in_=bass.AP(pt_all.tensor, iv * 8 + 4,
                                [[NT * 8, TILE], [1, 4]]))
                for j in range(1, K):
                    nc.gpsimd.indirect_dma_start(
                        out=gc[:, j, 0:4],
                        out_offset=None,
                        in_=table[:],
                        in_offset=bass.IndirectOffsetOnAxis(
                            ap=bass.AP(idx_all.tensor, iv * K + j,
                                       [[NT * K, TILE], [1, 1]]),
                            axis=0),
                    )

                # dist = (1+|q|^2) - 2 q.p + |p|^2, built in gc[:, :, 3]:
                #   sc    = q * (-2p)          (pt_all lanes 0:3 hold -2p)
                #   dist0 = sum_c sc           (reduce innermost)
                #   gc3   = (dist0 + |p|^2) + gc3
                gxyz = gc[:, :, 0:3]
                p_bc = bass.AP(pt_all.tensor, iv * 8,
                               [[NT * 8, TILE], [0, K], [1, 3]])
                sc = wpool.tile([TILE, K, 3], fp32, name="sc")
                nc.vector.tensor_tensor(
                    out=sc[:, :, :], in0=gxyz, in1=p_bc,
                    op=mybir.AluOpType.mult)
                dist0 = wpool.tile([TILE, K, 1], fp32, name="dist0")
                nc.vector.tensor_reduce(
                    out=dist0[:, :, :], in_=sc[:, :, :],
                    axis=mybir.AxisListType.X, op=mybir.AluOpType.add)
                nc.vector.scalar_tensor_tensor(
                    out=gc[:, :, 3:4], in0=dist0[:, :, :],
                    scalar=bass.AP(pt_all.tensor, iv * 8 + 3,
                                   [[NT * 8, TILE], [1, 1]]),
                    in1=gc[:, :, 3:4],
                    op0=mybir.AluOpType.add, op1=mybir.AluOpType.add)

                # einsum: contract (j, c) = 324 in three 108-row passes
                psum_o = opool.tile([OUT, TILE], fp32, name="psum_o")
                for p in range(3):
                    tp = ppool.tile([108, TILE], fp32, name="tp")
                    nc.tensor.transpose(
                        out=tp[:],
                        in_=bass.AP(gc.tensor, 108 * p, [[K * 4, TILE], [1, 108]]),
                        identity=ident[:])
                    mov = wpool.tile([108, TILE], fp32, name="mov")
                    nc.scalar.copy(out=mov[:], in_=tp[:])
                    nc.tensor.matmul(
                        psum_o[:], w2_tiles[p][:], mov[:],
                        start=(p == 0), stop=(p == 2))

                o_t = wpool.tile([OUT, TILE], fp32, name="o_t")
                nc.scalar.add(out=o_t[:], in_=psum_o[:], add=bias_t[:, 0:1])
                nc.sync.dma_start(out_d[iv * OUT:(iv + 1) * OUT], o_t[:])

    split_excess_waits(nc)
    return nc


_CACHED_NC = None


def kernel(points, indices, dw, weight, bias):
    global _CACHED_NC
    points = np.ascontiguousarray(points, dtype=np.float32)
    indices = np.ascontiguousarray(indices)
    dw = np.asarray(dw, dtype=np.float32)
    weight = np.asarray(weight, dtype=np.float32)
    bias = np.asarray(bias, dtype=np.float32)

    # Fold dw into the weights: W2[(j*4 + c), l] = dw[j, c] * weight[c, j, l]
    w2 = (dw[:, :, None] * weight.transpose(1, 0, 2)).reshape(324, OUT)
    w2 = np.ascontiguousarray(w2, dtype=np.float32)
    bias_col = np.ascontiguousarray(bias.reshape(OUT, 1))

    # gather table h = [x, y, z, 1 + |x|^2]
    sq = (points * points).sum(axis=1, keepdims=True)
    htab = np.ascontiguousarray(
        np.concatenate([points, 1.0 + sq], axis=1).astype(np.float32))
    # per-tile point features [-2x, -2y, -2z, |p|^2, x, y, z, 1+|p|^2]
    pfeat = np.concatenate(
        [-2.0 * points, sq, points, 1.0 + sq], axis=1).astype(np.float32)

    idx32 = indices.astype(np.int32)
    in_maps = []
    for c in range(NCORES):
        lo, hi = c * PPC, (c + 1) * PPC
        idx_pad = np.zeros((PADPC, K), dtype=np.int32)
        idx_pad[:PPC] = idx32[lo:hi]
        pt_pad = np.zeros((PADPC, 8), dtype=np.float32)
        pt_pad[:PPC] = pfeat[lo:hi]
        in_maps.append({
            "table": htab,
            "idx": idx_pad,
            "ptile": pt_pad,
            "w2": w2,
            "bias": bias_col,
        })

    global _last_in_maps
    _last_in_maps = in_maps
    if _CACHED_NC is None:
        _CACHED_NC = build_program()
    res = run_bass_kernel_spmd(_CACHED_NC, in_maps, core_ids=list(range(NCORES)))

    out = np.empty((N, OUT), dtype=np.float32)
    for c in range(NCORES):
        o = res.results[c]["out"].reshape(NT, OUT, TILE)
        o = o.transpose(0, 2, 1).reshape(PADPC, OUT)
        out[c * PPC:(c + 1) * PPC] = o[:PPC]
    return out


# revision 6
# speedup vs baseline: 1.0024x; 1.0024x over previous
"""Trainium2 Bass kernel for nn_ConvOnTree (gnn_message_passing).

Computation (reference):
    selected = points[indices]                      # [N, 81, 3]
    dist     = sum((selected - selected[:, :1])**2, -1) + 1
    data     = concat(selected, dist[..., None])    # [N, 81, 4]
    out      = einsum('njc,cjl->nl', dw * data, weight) + bias

Strategy: data-parallel over N across 8 NeuronCores. Each core holds a
host-precomputed feature table h = [x, y, z, 1+|x|^2] ([N, 4], 16B rows) in
HBM and gathers neighbor rows with per-tile vector-indirect DMAs (128 rows,
one per partition, per instruction — the SWDGE indirect1d ucode's hard limit;
multi-row-per-partition offset APs lower incorrectly and HWDGE engines cannot
execute the indirect opcode). j = 0 is the self neighbor and is copied from
the point tile instead of gathered. The distance feature is reconstructed
algebraically (dist = (1+|q|^2) - 2 q.p + |p|^2) in 3 DVE ops per tile using
a point tile that carries [-2x, -2y, -2z, |p|^2, x, y, z, 1+|p|^2]. The
einsum contracts (j, c) = 324 on PE in three 108-row passes with dw folded
into the weights on the host; bias is added on ACT. Output tiles are written
as [8, 128] and transposed on the host during unsharding.
"""
import sys
import types

sys.path.insert(0, "/opt/trn_rl_repo")
sys.path.insert(0, "/root/.axon_site")

import numpy as np
import concourse.bass as bass
import concourse.mybir as mybir
import concourse.tile as tile
from concourse.vector_clock import ScopedClock
from concourse.bass_utils import run_bass_kernel_spmd
from concourse.masks import make_identity

fp32 = mybir.dt.float32
i32 = mybir.dt.int32

N = 500000
K = 81
OUT = 8
NCORES = 8
PPC = N // NCORES            # 62500 points per core
TILE = 128
NT = (PPC + TILE - 1) // TILE  # 489 tiles per core
PADPC = NT * TILE            # 62592 padded points per core


def _patched_drain_and_barrier(self, tick_clock, wait_clock):
    # This walrus build's CTRL_NO struct accepts too few sync waits for the
    # tile tail drain; spread the waits across preceding SP nops.
    nops = [self.nc.sync.nop() for _ in range(30)]
    drain_inst = self.nc.sync.drain()
    wait_clock.add_sem_waits(
        drain_inst.ins, ScopedClock({None: tick_clock.global_clock})
    )
    waits = list(drain_inst.ins.sync_info.on_wait) if drain_inst.ins.sync_info else []
    if len(waits) > 1:
        drain_inst.ins.sync_info.on_wait = waits[:1]
        for w, nop in zip(waits[1:], nops):
            si = nop.ins.sync_info
            if si is None:
                nop.ins.sync_info = mybir.SyncInfo(on_wait=[w], on_update=[])
            else:
                si.on_wait.append(w)
    self.nc.all_engine_barrier()
    popped = self.nc._tile_sem_poison_stack.pop()
    assert popped is self._sem_poison
    self.nc.clear_and_free_semaphores(list(self.sems.allocated().values()))
    self.nc.all_engine_barrier()


tile.TileContext._drain_and_barrier = _patched_drain_and_barrier


def _install_ntff_hook():
    # The image's antenv lacks axon_hooks; register the ctypes NTFF hook so
    # trace=True can report HW exec time. Harmless if tracing is never used.
    try:
        from trn_agent_boot.trn_boot import _ntff_profile_via_ctypes

        hook = _ntff_profile_via_ctypes("/opt/axon/libaxon_pjrt.so")
        mod = types.ModuleType("antenv.axon_hooks")
        mod.get_axon_ntff_profile_hook = lambda: hook
        import antenv  # noqa: F401

        sys.modules["antenv.axon_hooks"] = mod
    except Exception:
        pass


_install_ntff_hook()


MAX_WAITS = 1  # this walrus build encodes only one sync wait per instruction


def split_excess_waits(nc):
    """Move sync waits beyond MAX_WAITS onto same-engine InstNoOp carriers
    inserted immediately before the over-limit instruction."""
    n_split = 0
    for f in nc.m.functions:
        for b in f.blocks:
            new_insts = []
            for inst in b.instructions:
                si = inst.sync_info
                if si is not None and si.on_wait and len(si.on_wait) > MAX_WAITS:
                    waits = list(si.on_wait)
                    for k, w in enumerate(waits[MAX_WAITS:]):
                        nop = mybir.InstNoOp(
                            name=f"{inst.name}-wsplit{k}", ins=[], outs=[])
                        nop.engine = inst.engine
                        nop.sync_info = mybir.SyncInfo(on_wait=[w], on_update=[])
                        new_insts.append(nop)
                        n_split += 1
                    si.on_wait = waits[:MAX_WAITS]
                new_insts.append(inst)
            if len(new_insts) != len(b.instructions):
                b.instructions[:] = new_insts
    return n_split


def build_program():
    nc = bass.Bass("TRN2", target_bir_lowering=False, debug=False,
                   num_devices=NCORES)
    # h table: [x, y, z, 1 + |x|^2] per point, 16B rows
    table = nc.dram_tensor("table", [N, 4], fp32, kind="ExternalInput")
    idx_in = nc.dram_tensor("idx", [NT * TILE, K], i32, kind="ExternalInput")
    # per-tile points: [-2x, -2y, -2z, |p|^2, x, y, z, 1+|p|^2]
    pt_in = nc.dram_tensor("ptile", [NT * TILE, 8], fp32, kind="ExternalInput")
    w2_in = nc.dram_tensor("w2", [324, OUT], fp32, kind="ExternalInput")
    bias_in = nc.dram_tensor("bias", [OUT, 1], fp32, kind="ExternalInput")
    out_d = nc.dram_tensor("out", [NT * OUT, TILE], fp32, kind="ExternalOutput")

    with tile.TileContext(nc) as tc:
        with (
            tc.tile_pool(name="const", bufs=1) as cpool,
            tc.tile_pool(name="work", bufs=6) as wpool,
            tc.tile_pool(name="ps", bufs=3, space="PSUM") as ppool,
            tc.tile_pool(name="pso", bufs=2, space="PSUM") as opool,
        ):
            ident = cpool.tile([128, 128], fp32, name="ident")
            make_identity(nc, ident[:])
            w2_tiles = []
            for p in range(3):
                w2p = cpool.tile([108, OUT], fp32, name=f"w2_{p}")
                nc.sync.dma_start(w2p[:], w2_in[108 * p:108 * (p + 1), :])
                w2_tiles.append(w2p)
            bias_t = cpool.tile([OUT, 1], fp32, name="bias_t")
            nc.sync.dma_start(bias_t[:], bias_in[:])

            for iv in range(NT):
                idx_t = wpool.tile([TILE, K], i32, name="idx_t")
                nc.sync.dma_start(idx_t[:], idx_in[iv * TILE:(iv + 1) * TILE])
                p_t = wpool.tile([TILE, 8], fp32, name="p_t")
                nc.sync.dma_start(p_t[:], pt_in[iv * TILE:(iv + 1) * TILE])

                # gathered tile: [128 pts, 81 nbrs, 4] = [x, y, z, 1+|q|^2]
                gc = wpool.tile([TILE, K, 4], fp32, name="gc")
                # j = 0 is the self neighbor: copy the h-row from p_t.
                nc.vector.tensor_copy(out=gc[:, 0, 0:4], in_=p_t[:, 4:8])
                for j in range(1, K):
                    nc.gpsimd.indirect_dma_start(
                        out=gc[:, j, 0:4],
                        out_offset=None,
                        in_=table[:],
                        in_offset=bass.IndirectOffsetOnAxis(
                            ap=idx_t[:, j:j + 1], axis=0),
                    )

                # dist = (1+|q|^2) - 2 q.p + |p|^2, built in gc[:, :, 3]:
                #   sc    = q * (-2p)          (p_t lanes 0:3 hold -2p)
                #   dist0 = sum_c sc           (reduce innermost)
                #   gc3   = (dist0 + |p|^2) + gc3
                gxyz = gc[:, :, 0:3]
                p_bc = bass.AP(p_t.tensor, 0, [[8, TILE], [0, K], [1, 3]])
                sc = wpool.tile([TILE, K, 3], fp32, name="sc")
                nc.vector.tensor_tensor(
                    out=sc[:, :, :], in0=gxyz, in1=p_bc,
                    op=mybir.AluOpType.mult)
                dist0 = wpool.tile([TILE, K, 1], fp32, name="dist0")
                nc.vector.tensor_reduce(
                    out=dist0[:, :, :], in_=sc[:, :, :],
                    axis=mybir.AxisListType.X, op=mybir.AluOpType.add)
                nc.vector.scalar_tensor_tensor(
                    out=gc[:, :, 3:4], in0=dist0[:, :, :],
                    scalar=p_t[:, 3:4], in1=gc[:, :, 3:4],
                    op0=mybir.AluOpType.add, op1=mybir.AluOpType.add)

                # einsum: contract (j, c) = 324 in three 108-row passes
                psum_o = opool.tile([OUT, TILE], fp32, name="psum_o")
                for p in range(3):
                    tp = ppool.tile([108, TILE], fp32, name="tp")
                    nc.tensor.transpose(
                        out=tp[:],
                        in_=bass.AP(gc.tensor, 108 * p, [[K * 4, TILE], [1, 108]]),
                        identity=ident[:])
                    mov = wpool.tile([108, TILE], fp32, name="mov")
                    nc.scalar.copy(out=mov[:], in_=tp[:])
                    nc.tensor.matmul(
                        psum_o[:], w2_tiles[p][:], mov[:],
                        start=(p == 0), stop=(p == 2))

                o_t = wpool.tile([OUT, TILE], fp32, name="o_t")
                nc.scalar.add(out=o_t[:], in_=psum_o[:], add=bias_t[:, 0:1])
                nc.sync.dma_start(out_d[iv * OUT:(iv + 1) * OUT], o_t[:])

    split_excess_waits(nc)
    return nc


_CACHED_NC = None


def kernel(points, indices, dw, weight, bias):
    global _CACHED_NC
    points = np.ascontiguousarray(points, dtype=np.float32)
    indices = np.ascontiguousarray(indices)
    dw = np.asarray(dw, dtype=np.float32)
    weight = np.asarray(weight, dtype=np.float32)
    bias = np.asarray(bias, dtype=np.float32)

    # Fold dw into the weights: W2[(j*4 + c), l] = dw[j, c] * weight[c, j, l]
    w2 = (dw[:, :, None] * weight.transpose(1, 0, 2)).reshape(324, OUT)
    w2 = np.ascontiguousarray(w2, dtype=np.float32)
    bias_col = np.ascontiguousarray(bias.reshape(OUT, 1))

    # gather table h = [x, y, z, 1 + |x|^2]
    sq = (points * points).sum(axis=1, keepdims=True)
    htab = np.ascontiguousarray(
        np.concatenate([points, 1.0 + sq], axis=1).astype(np.float32))
    # per-tile point features [-2x, -2y, -2z, |p|^2, x, y, z, 1+|p|^2]
    pfeat = np.concatenate(
        [-2.0 * points, sq, points, 1.0 + sq], axis=1).astype(np.float32)

    idx32 = indices.astype(np.int32)
    in_maps = []
    for c in range(NCORES):
        lo, hi = c * PPC, (c + 1) * PPC
        idx_pad = np.zeros((PADPC, K), dtype=np.int32)
        idx_pad[:PPC] = idx32[lo:hi]
        pt_pad = np.zeros((PADPC, 8), dtype=np.float32)
        pt_pad[:PPC] = pfeat[lo:hi]
        in_maps.append({
            "table": htab,
            "idx": idx_pad,
            "ptile": pt_pad,
            "w2": w2,
            "bias": bias_col,
        })

    global _last_in_maps
    _last_in_maps = in_maps
    if _CACHED_NC is None:
        _CACHED_NC = build_program()
    res = run_bass_kernel_spmd(_CACHED_NC, in_maps, core_ids=list(range(NCORES)))

    out = np.empty((N, OUT), dtype=np.float32)
    for c in range(NCORES):
        o = res.results[c]["out"].reshape(NT, OUT, TILE)
        o = o.transpose(0, 2, 1).reshape(PADPC, OUT)
        out[c * PPC:(c + 1) * PPC] = o[:PPC]
    return out


# revision 8
# speedup vs baseline: 1.0093x; 1.0069x over previous
"""Trainium2 Bass kernel for nn_ConvOnTree (gnn_message_passing).

Computation (reference):
    selected = points[indices]                      # [N, 81, 3]
    dist     = sum((selected - selected[:, :1])**2, -1) + 1
    data     = concat(selected, dist[..., None])    # [N, 81, 4]
    out      = einsum('njc,cjl->nl', dw * data, weight) + bias

Strategy: data-parallel over N across 8 NeuronCores. Each core holds a
host-precomputed feature table h = [x, y, z, 1+|x|^2] ([N, 4], 16B rows) in
HBM and gathers neighbor rows with per-tile vector-indirect DMAs (128 rows,
one per partition, per instruction — the SWDGE indirect1d ucode's hard limit;
multi-row-per-partition offset APs lower incorrectly and HWDGE engines cannot
execute the indirect opcode). j = 0 is the self neighbor and is copied from
the point tile instead of gathered. The distance feature is reconstructed
algebraically (dist = (1+|q|^2) - 2 q.p + |p|^2) in 3 DVE ops per tile using
a point tile that carries [-2x, -2y, -2z, |p|^2, x, y, z, 1+|p|^2]. The
einsum contracts (j, c) = 324 on PE in three 108-row passes with dw folded
into the weights on the host; bias is added on ACT. Output tiles are written
as [8, 128] and transposed on the host during unsharding.
"""
import sys
import types

sys.path.insert(0, "/opt/trn_rl_repo")
sys.path.insert(0, "/root/.axon_site")

import numpy as np
import concourse.bass as bass
import concourse.mybir as mybir
import concourse.tile as tile
from concourse.vector_clock import ScopedClock
from concourse.bass_utils import run_bass_kernel_spmd
from concourse.masks import make_identity

fp32 = mybir.dt.float32
i32 = mybir.dt.int32

N = 500000
K = 81
OUT = 8
NCORES = 8
PPC = N // NCORES            # 62500 points per core
TILE = 128
NT = (PPC + TILE - 1) // TILE  # 489 tiles per core
PADPC = NT * TILE            # 62592 padded points per core


def _patched_drain_and_barrier(self, tick_clock, wait_clock):
    # This walrus build's CTRL_NO struct accepts too few sync waits for the
    # tile tail drain; spread the waits across preceding SP nops.
    nops = [self.nc.sync.nop() for _ in range(30)]
    drain_inst = self.nc.sync.drain()
    wait_clock.add_sem_waits(
        drain_inst.ins, ScopedClock({None: tick_clock.global_clock})
    )
    waits = list(drain_inst.ins.sync_info.on_wait) if drain_inst.ins.sync_info else []
    if len(waits) > 1:
        drain_inst.ins.sync_info.on_wait = waits[:1]
        for w, nop in zip(waits[1:], nops):
            si = nop.ins.sync_info
            if si is None:
                nop.ins.sync_info = mybir.SyncInfo(on_wait=[w], on_update=[])
            else:
                si.on_wait.append(w)
    self.nc.all_engine_barrier()
    popped = self.nc._tile_sem_poison_stack.pop()
    assert popped is self._sem_poison
    self.nc.clear_and_free_semaphores(list(self.sems.allocated().values()))
    self.nc.all_engine_barrier()


tile.TileContext._drain_and_barrier = _patched_drain_and_barrier


def _install_ntff_hook():
    # The image's antenv lacks axon_hooks; register the ctypes NTFF hook so
    # trace=True can report HW exec time. Harmless if tracing is never used.
    try:
        from trn_agent_boot.trn_boot import _ntff_profile_via_ctypes

        hook = _ntff_profile_via_ctypes("/opt/axon/libaxon_pjrt.so")
        mod = types.ModuleType("antenv.axon_hooks")
        mod.get_axon_ntff_profile_hook = lambda: hook
        import antenv  # noqa: F401

        sys.modules["antenv.axon_hooks"] = mod
    except Exception:
        pass


_install_ntff_hook()


MAX_WAITS = 1  # this walrus build encodes only one sync wait per instruction


def split_excess_waits(nc):
    """Move sync waits beyond MAX_WAITS onto same-engine InstNoOp carriers
    inserted immediately before the over-limit instruction."""
    n_split = 0
    for f in nc.m.functions:
        for b in f.blocks:
            new_insts = []
            for inst in b.instructions:
                si = inst.sync_info
                if si is not None and si.on_wait and len(si.on_wait) > MAX_WAITS:
                    waits = list(si.on_wait)
                    for k, w in enumerate(waits[MAX_WAITS:]):
                        nop = mybir.InstNoOp(
                            name=f"{inst.name}-wsplit{k}", ins=[], outs=[])
                        nop.engine = inst.engine
                        nop.sync_info = mybir.SyncInfo(on_wait=[w], on_update=[])
                        new_insts.append(nop)
                        n_split += 1
                    si.on_wait = waits[:MAX_WAITS]
                new_insts.append(inst)
            if len(new_insts) != len(b.instructions):
                b.instructions[:] = new_insts
    return n_split


def build_program():
    nc = bass.Bass("TRN2", target_bir_lowering=False, debug=False,
                   num_devices=NCORES)
    # h table: [x, y, z, 1 + |x|^2] per point, 16B rows
    table = nc.dram_tensor("table", [N, 4], fp32, kind="ExternalInput")
    idx_in = nc.dram_tensor("idx", [NT * TILE, K], i32, kind="ExternalInput")
    # per-tile points: [-2x, -2y, -2z, |p|^2, x, y, z, 1+|p|^2]
    pt_in = nc.dram_tensor("ptile", [NT * TILE, 8], fp32, kind="ExternalInput")
    w2_in = nc.dram_tensor("w2", [324, OUT], fp32, kind="ExternalInput")
    bias_in = nc.dram_tensor("bias", [OUT, 1], fp32, kind="ExternalInput")
    out_d = nc.dram_tensor("out", [NT * OUT, TILE], fp32, kind="ExternalOutput")

    with tile.TileContext(nc) as tc:
        with (
            tc.tile_pool(name="const", bufs=1) as cpool,
            tc.tile_pool(name="work", bufs=6) as wpool,
            tc.tile_pool(name="ps", bufs=3, space="PSUM") as ppool,
            tc.tile_pool(name="pso", bufs=2, space="PSUM") as opool,
        ):
            ident = cpool.tile([128, 128], fp32, name="ident")
            make_identity(nc, ident[:])
            w2_tiles = []
            for p in range(3):
                w2p = cpool.tile([108, OUT], fp32, name=f"w2_{p}")
                nc.sync.dma_start(w2p[:], w2_in[108 * p:108 * (p + 1), :])
                w2_tiles.append(w2p)
            bias_t = cpool.tile([OUT, 1], fp32, name="bias_t")
            nc.sync.dma_start(bias_t[:], bias_in[:])

            for iv in range(NT):
                idx_t = wpool.tile([TILE, K], i32, name="idx_t")
                nc.sync.dma_start(idx_t[:], idx_in[iv * TILE:(iv + 1) * TILE])
                p_t = wpool.tile([TILE, 8], fp32, name="p_t")
                nc.sync.dma_start(p_t[:], pt_in[iv * TILE:(iv + 1) * TILE])

                # gathered tile: [128 pts, 81 nbrs, 4] = [x, y, z, 1+|q|^2]
                gc = wpool.tile([TILE, K, 4], fp32, name="gc")
                # j = 0 is the self neighbor: copy the h-row from p_t.
                # tensor_tensor(max, x, x) == copy, but unlike tensor_copy it
                # can never enter DVE 2-port perf mode, which would lock
                # GpSimd out of SBUF and stall SWDGE descriptor generation.
                nc.vector.tensor_tensor(
                    out=gc[:, 0, 0:4], in0=p_t[:, 4:8], in1=p_t[:, 4:8],
                    op=mybir.AluOpType.max)
                for j in range(1, K):
                    nc.gpsimd.indirect_dma_start(
                        out=gc[:, j, 0:4],
                        out_offset=None,
                        in_=table[:],
                        in_offset=bass.IndirectOffsetOnAxis(
                            ap=idx_t[:, j:j + 1], axis=0),
                    )

                # dist = (1+|q|^2) - 2 q.p + |p|^2, built in gc[:, :, 3]:
                #   sc    = q * (-2p)          (p_t lanes 0:3 hold -2p)
                #   dist0 = sum_c sc           (reduce innermost)
                #   gc3   = (dist0 + |p|^2) + gc3
                gxyz = gc[:, :, 0:3]
                p_bc = bass.AP(p_t.tensor, 0, [[8, TILE], [0, K], [1, 3]])
                sc = wpool.tile([TILE, K, 3], fp32, name="sc")
                nc.vector.tensor_tensor(
                    out=sc[:, :, :], in0=gxyz, in1=p_bc,
                    op=mybir.AluOpType.mult)
                dist0 = wpool.tile([TILE, K, 1], fp32, name="dist0")
                nc.vector.tensor_reduce(
                    out=dist0[:, :, :], in_=sc[:, :, :],
                    axis=mybir.AxisListType.X, op=mybir.AluOpType.add)
                # two tensor_tensor adds instead of scalar_tensor_tensor: the
                # tensor_scalar family can enter DVE 2-port mode and block
                # GpSimd (SWDGE) SBUF access; tensor_tensor cannot.
                nc.vector.tensor_tensor(
                    out=gc[:, :, 3:4], in0=dist0[:, :, :],
                    in1=gc[:, :, 3:4], op=mybir.AluOpType.add)
                pp_bc = bass.AP(p_t.tensor, 3, [[8, TILE], [0, K], [1, 1]])
                nc.vector.tensor_tensor(
                    out=gc[:, :, 3:4], in0=gc[:, :, 3:4],
                    in1=pp_bc, op=mybir.AluOpType.add)

                # einsum: contract (j, c) = 324 in three 108-row passes
                psum_o = opool.tile([OUT, TILE], fp32, name="psum_o")
                for p in range(3):
                    tp = ppool.tile([108, TILE], fp32, name="tp")
                    nc.tensor.transpose(
                        out=tp[:],
                        in_=bass.AP(gc.tensor, 108 * p, [[K * 4, TILE], [1, 108]]),
                        identity=ident[:])
                    mov = wpool.tile([108, TILE], fp32, name="mov")
                    nc.scalar.copy(out=mov[:], in_=tp[:])
                    nc.tensor.matmul(
                        psum_o[:], w2_tiles[p][:], mov[:],
                        start=(p == 0), stop=(p == 2))

                o_t = wpool.tile([OUT, TILE], fp32, name="o_t")
                nc.scalar.add(out=o_t[:], in_=psum_o[:], add=bias_t[:, 0:1])
                nc.sync.dma_start(out_d[iv * OUT:(iv + 1) * OUT], o_t[:])

    split_excess_waits(nc)
    return nc


_CACHED_NC = None


def kernel(points, indices, dw, weight, bias):
    global _CACHED_NC
    points = np.ascontiguousarray(points, dtype=np.float32)
    indices = np.ascontiguousarray(indices)
    dw = np.asarray(dw, dtype=np.float32)
    weight = np.asarray(weight, dtype=np.float32)
    bias = np.asarray(bias, dtype=np.float32)

    # Fold dw into the weights: W2[(j*4 + c), l] = dw[j, c] * weight[c, j, l]
    w2 = (dw[:, :, None] * weight.transpose(1, 0, 2)).reshape(324, OUT)
    w2 = np.ascontiguousarray(w2, dtype=np.float32)
    bias_col = np.ascontiguousarray(bias.reshape(OUT, 1))

    # gather table h = [x, y, z, 1 + |x|^2]
    sq = (points * points).sum(axis=1, keepdims=True)
    htab = np.ascontiguousarray(
        np.concatenate([points, 1.0 + sq], axis=1).astype(np.float32))
    # per-tile point features [-2x, -2y, -2z, |p|^2, x, y, z, 1+|p|^2]
    pfeat = np.concatenate(
        [-2.0 * points, sq, points, 1.0 + sq], axis=1).astype(np.float32)

    idx32 = indices.astype(np.int32)
    in_maps = []
    for c in range(NCORES):
        lo, hi = c * PPC, (c + 1) * PPC
        idx_pad = np.zeros((PADPC, K), dtype=np.int32)
        idx_pad[:PPC] = idx32[lo:hi]
        pt_pad = np.zeros((PADPC, 8), dtype=np.float32)
        pt_pad[:PPC] = pfeat[lo:hi]
        in_maps.append({
            "table": htab,
            "idx": idx_pad,
            "ptile": pt_pad,
            "w2": w2,
            "bias": bias_col,
        })

    global _last_in_maps
    _last_in_maps = in_maps
    if _CACHED_NC is None:
        _CACHED_NC = build_program()
    res = run_bass_kernel_spmd(_CACHED_NC, in_maps, core_ids=list(range(NCORES)))

    out = np.empty((N, OUT), dtype=np.float32)
    for c in range(NCORES):
        o = res.results[c]["out"].reshape(NT, OUT, TILE)
        o = o.transpose(0, 2, 1).reshape(PADPC, OUT)
        out[c * PPC:(c + 1) * PPC] = o[:PPC]
    return out
